# revision 10
# baseline (speedup 1.0000x reference)
"""Trainium2 Bass kernel for nn_Attention_Decoder (8-core tensor-parallel).

Sharding:
  - emb / lo_w / lo_b: vocab-sharded (V8=6656 rows/core, padded).
  - att_w / att_b / encoder_outs: L-sharded (256 rows/core), softmax via
    log-sum-exp package AllGather.
  - GRU: output-sharded (64 h-elems/core), per-layer AllGather of h chunk.
  - conv/BN stack: replicated on every core (tiny).
Collectives (all 8-core AllGather): emb row, attention package, 4x GRU xt,
logit stats.
"""
import sys
import numpy as np

sys.path.insert(0, "/opt/trn_rl_repo")

import concourse.bass as bass
import concourse.bacc as bacc
import concourse.tile as tile
from concourse import mybir
from concourse.bass_utils import run_bass_kernel_spmd

F32 = mybir.dt.float32
I32 = mybir.dt.int32
AX = mybir.AxisListType
ALU = mybir.AluOpType
ACTF = mybir.ActivationFunctionType

NCORE = 8
V = 50257
E = H = 512
LATT = 2048
V8 = 6656            # padded vocab rows per core
VT = V8 // 128       # 52
LS = LATT // NCORE   # 256
G = H // NCORE       # 64
EPS = 1e-5
NEGB = -1e30
RG = [[0, 1, 2, 3, 4, 5, 6, 7]]


def build_nc():
    nc = bacc.Bacc("TRN2", target_bir_lowering=False, debug=False,
                   num_devices=NCORE)

    def din(name, shape, dtype=F32):
        return nc.dram_tensor(name, list(shape), dtype, kind="ExternalInput")

    # ---- inputs (per-core values supplied in in_maps) ----
    idx_i = din("idx_i", (2, 1), I32)        # clamp(x - c*V8, 0, V8-1), dup
    own_i = din("own_i", (2, 1), I32)        # owner core = x // V8, dup
    emb_i = din("emb_i", (V8, E))            # emb vocab shard
    h_i = din("h_i", (4, H))                 # h_state rows
    hT_i = din("hT_i", (128, 16))            # [p, kc*4+l] = h[l, kc*128+p]
    hprev_i = din("hprev_i", (1, 4 * G))     # [l*G+j] = h[l, c*G+j]
    cw1_i = din("cw1_i", (5, 3 * 32))
    cw2_i = din("cw2_i", (32, 3 * 5))
    cw3_i = din("cw3_i", (5, 3))
    cw4_i = din("cw4_i", (3, 3 * 32))
    cw5_i = din("cw5_i", (32, 3 * 3))
    cw6_i = din("cw6_i", (3, 3))
    bng1_i = din("bng1_i", (32, 1)); bnb1_i = din("bnb1_i", (32, 1))
    bng2_i = din("bng2_i", (5, 1)); bnb2_i = din("bnb2_i", (5, 1))
    bng3_i = din("bng3_i", (1, 1)); bnb3_i = din("bnb3_i", (1, 1))
    bng4_i = din("bng4_i", (32, 1)); bnb4_i = din("bnb4_i", (32, 1))
    bng5_i = din("bng5_i", (3, 1)); bnb5_i = din("bnb5_i", (3, 1))
    bng6_i = din("bng6_i", (1, 1)); bnb6_i = din("bnb6_i", (1, 1))
    attw_i = din("attw_i", (128, 4 * LS))    # [p, kc*LS+j] = att_w[c*LS+j, kc*128+p]
    attb_i = din("attb_i", (1, LS))
    enc_i = din("enc_i", (LS, 2 * H))        # encoder_outs shard rows
    wih_i = din("wih_i", (128, 16 * 192))    # [p,(l*4+kc)*192+r]
    whh_i = din("whh_i", (128, 16 * 192))
    bih_i = din("bih_i", (1, 4 * 192))
    bhh_i = din("bhh_i", (1, 4 * 192))
    low_i = din("low_i", (128, VT * 512))    # [k,(vt*4+kc)*128+m] = W[vt*128+m, kc*128+k]
    lob_i = din("lob_i", (128, VT))          # [p, vt] = b[vt*128+p]
    idt_i = din("idt_i", (128, 128))

    out_lp = nc.dram_tensor("out_lp", [128, VT], F32, kind="ExternalOutput")
    out_h = nc.dram_tensor("out_h", [4, H], F32, kind="ExternalOutput")

    with tile.TileContext(nc) as tc, \
         tc.tile_pool(name="w", bufs=1) as wp, \
         tc.tile_pool(name="s", bufs=1) as sp, \
         tc.tile_pool(name="ppA", bufs=3, space="PSUM") as ppA, \
         tc.tile_pool(name="ppT", bufs=1, space="PSUM") as ppT, \
         tc.tile_pool(name="ppB", bufs=1, space="PSUM") as ppB, \
         tc.tile_pool(name="ppLO", bufs=1, space="PSUM") as ppLO, \
         tc.tile_pool(name="dram", bufs=1, space="DRAM") as dp:

        dma = nc.sync.dma_start

        def wtile(src, shape, tag):
            t = wp.tile(list(shape), F32, tag=tag)
            dma(t[:], src.ap())
            return t

        # ---- load small weights ----
        idt = wtile(idt_i, (128, 128), "idt")
        h_sb = wtile(h_i, (4, H), "h_sb")
        hT = wtile(hT_i, (128, 16), "hT")
        hprev = wtile(hprev_i, (1, 4 * G), "hprev")
        cw1 = wtile(cw1_i, (5, 3, 32), "cw1")
        cw2 = wtile(cw2_i, (32, 3, 5), "cw2")
        cw3 = wtile(cw3_i, (5, 3), "cw3")
        cw4 = wtile(cw4_i, (3, 3, 32), "cw4")
        cw5 = wtile(cw5_i, (32, 3, 3), "cw5")
        cw6 = wtile(cw6_i, (3, 3), "cw6")
        bng = {}
        for nm, hd, c in [("g1", bng1_i, 32), ("b1", bnb1_i, 32),
                          ("g2", bng2_i, 5), ("b2", bnb2_i, 5),
                          ("g3", bng3_i, 1), ("b3", bnb3_i, 1),
                          ("g4", bng4_i, 32), ("b4", bnb4_i, 32),
                          ("g5", bng5_i, 3), ("b5", bnb5_i, 3),
                          ("g6", bng6_i, 1), ("b6", bnb6_i, 1)]:
            bng[nm] = wtile(hd, (c, 1), "bn" + nm)
        attw = wtile(attw_i, (128, 4, LS), "attw")
        attb = wtile(attb_i, (1, LS), "attb")
        enc0 = wp.tile([128, 2 * H], F32, tag="enc0")
        enc1 = wp.tile([128, 2 * H], F32, tag="enc1")
        dma(enc0[:], enc_i.ap()[0:128, :])
        dma(enc1[:], enc_i.ap()[128:256, :])
        wih = wtile(wih_i, (128, 4, 4, 192), "wih")
        whh = wtile(whh_i, (128, 4, 4, 192), "whh")
        bih = wtile(bih_i, (1, 4, 192), "bih")
        bhh = wtile(bhh_i, (1, 4, 192), "bhh")
        lob = wtile(lob_i, (128, VT), "lob")
        low = wp.tile([128, VT * 512], F32, tag="low")
        for g in range(13):
            dma(low[:, g * 2048:(g + 1) * 2048],
                low_i.ap()[:, g * 2048:(g + 1) * 2048])

        ones_row = wp.tile([1, 128], F32, tag="ones_row")
        nc.vector.memset(ones_row[:], 1.0)

        # ---- helpers ----
        def bc128(src11, tag, npart=128):
            """materialize [npart,1] broadcast of a [1,1] sbuf value via PE"""
            ps = ppT.tile([npart, 1], F32, tag="ppT")
            nc.tensor.matmul(ps[:], ones_row[0:1, 0:npart], src11,
                             start=True, stop=True)
            sb = sp.tile([npart, 1], F32, tag=tag)
            nc.vector.tensor_copy(sb[:], ps[:])
            return sb

        def part_reduce(col_sb, op, tag, npart=128, negate=False):
            """reduce a [npart,1] sbuf column across partitions -> [1,1] sbuf"""
            ps = ppT.tile([1, npart], F32, tag="ppT")
            nc.tensor.transpose(ps[:], col_sb, idt[0:npart, 0:npart])
            out = sp.tile([1, 1], F32, tag=tag)
            nc.vector.tensor_reduce(out[:], ps[:], axis=AX.X, op=op,
                                    negate=negate)
            return out

        def conv3tap(psum, xpad, w_sb, O):
            for k in range(3):
                nc.tensor.matmul(psum[0:O, :], w_sb[:, k, :],
                                 xpad[:, k:k + 512],
                                 start=(k == 0), stop=(k == 2))

        def convT(psum128x4, xpad, w_sb):
            for c4 in range(4):
                for k in range(3):
                    nc.tensor.matmul(psum128x4[:, c4:c4 + 1],
                                     xpad[:, c4 * 128 + k:c4 * 128 + k + 128],
                                     w_sb[:, k:k + 1],
                                     start=(k == 0), stop=(k == 2))

        def bn_relu(src, C, g_sb, b_sb, out_ap, uid):
            """src [C,512] (psum) -> out_ap [C,512] (sbuf), training-mode BN."""
            sq = sp.tile([32, 512], F32, tag="bn_sq")
            sumsq = sp.tile([32, 1], F32, tag="bn_ss")
            sx = sp.tile([32, 1], F32, tag="bn_sx")
            nc.scalar.activation(sq[0:C, :], src, ACTF.Square,
                                 accum_out=sumsq[0:C, :])
            nc.vector.tensor_reduce(sx[0:C, :], src, axis=AX.X, op=ALU.add)
            mean = sp.tile([32, 1], F32, tag="bn_m")
            ex2 = sp.tile([32, 1], F32, tag="bn_e")
            nc.vector.tensor_scalar(mean[0:C, :], sx[0:C, :], 1.0 / 512, None,
                                    ALU.mult)
            nc.vector.tensor_scalar(ex2[0:C, :], sumsq[0:C, :], 1.0 / 512,
                                    None, ALU.mult)
            var = sp.tile([32, 1], F32, tag="bn_v")
            m2 = sp.tile([32, 1], F32, tag="bn_m2")
            nc.vector.tensor_mul(m2[0:C, :], mean[0:C, :], mean[0:C, :])
            nc.vector.tensor_sub(var[0:C, :], ex2[0:C, :], m2[0:C, :])
            nc.vector.tensor_scalar(var[0:C, :], var[0:C, :], EPS, None,
                                    ALU.add)
            std = sp.tile([32, 1], F32, tag="bn_st")
            nc.scalar.activation(std[0:C, :], var[0:C, :], ACTF.Sqrt)
            rstd = sp.tile([32, 1], F32, tag="bn_rs")
            nc.vector.reciprocal(rstd[0:C, :], std[0:C, :])
            s_ = sp.tile([32, 1], F32, tag="bn_s_")
            t_ = sp.tile([32, 1], F32, tag="bn_t_")
            nc.vector.tensor_mul(s_[0:C, :], rstd[0:C, :], g_sb)
            nc.vector.tensor_mul(t_[0:C, :], mean[0:C, :], s_[0:C, :])
            nc.vector.tensor_sub(t_[0:C, :], b_sb, t_[0:C, :])
            nc.scalar.activation(out_ap, src, ACTF.Relu,
                                 bias=t_[0:C, :], scale=s_[0:C, :])

        def bn_relu_T(psum, g11, b11, out_sb, uid):
            """src psum [128,4] (512 vals), BN over all, relu -> out_sb [128,4]"""
            sq = sp.tile([128, 4], F32, tag="bnT_sq")
            sumsq = sp.tile([128, 1], F32, tag="bnT_ss")
            sx = sp.tile([128, 1], F32, tag="bnT_sx")
            nc.scalar.activation(sq[:], psum[:], ACTF.Square,
                                 accum_out=sumsq[:])
            nc.vector.tensor_reduce(sx[:], psum[:], axis=AX.X, op=ALU.add)
            sxs = part_reduce(sx[:], ALU.add, "bnT_sxs_" + uid)
            sqs = part_reduce(sumsq[:], ALU.add, "bnT_sqs_" + uid)
            mean = sp.tile([1, 1], F32, tag="bnT_m")
            ex2 = sp.tile([1, 1], F32, tag="bnT_e")
            nc.vector.tensor_scalar(mean[:], sxs[:], 1.0 / 512, None, ALU.mult)
            nc.vector.tensor_scalar(ex2[:], sqs[:], 1.0 / 512, None, ALU.mult)
            m2 = sp.tile([1, 1], F32, tag="bnT_m2")
            var = sp.tile([1, 1], F32, tag="bnT_v")
            nc.vector.tensor_mul(m2[:], mean[:], mean[:])
            nc.vector.tensor_sub(var[:], ex2[:], m2[:])
            nc.vector.tensor_scalar(var[:], var[:], EPS, None, ALU.add)
            std = sp.tile([1, 1], F32, tag="bnT_st")
            nc.scalar.activation(std[:], var[:], ACTF.Sqrt)
            rstd = sp.tile([1, 1], F32, tag="bnT_rs")
            nc.vector.reciprocal(rstd[:], std[:])
            s11 = sp.tile([1, 1], F32, tag="bnT_s_")
            t11 = sp.tile([1, 1], F32, tag="bnT_t_")
            nc.vector.tensor_mul(s11[:], rstd[:], g11)
            nc.vector.tensor_mul(t11[:], mean[:], s11[:])
            nc.vector.tensor_sub(t11[:], b11, t11[:])
            s_bc = bc128(s11[:], "bnT_sbc_" + uid)
            t_bc = bc128(t11[:], "bnT_tbc_" + uid)
            nc.scalar.activation(out_sb[:], psum[:], ACTF.Relu,
                                 bias=t_bc[:], scale=s_bc[:])

        # ================= emb gather + AllGather =================
        idx_sb = sp.tile([2, 1], I32, tag="idx_sb")
        own_sb = sp.tile([2, 1], I32, tag="own_sb")
        dma(idx_sb[:], idx_i.ap())
        dma(own_sb[:], own_i.ap())
        row_sb = sp.tile([2, E], F32, tag="row_sb")
        nc.gpsimd.indirect_dma_start(
            row_sb[:], None, emb_i.ap(),
            bass.IndirectOffsetOnAxis(ap=idx_sb[0:2, 0:1], axis=0))
        embag_in = dp.tile([E], F32, tag="embag_in")
        embag_out = dp.tile([NCORE * E], F32, tag="embag_out")
        nc.gpsimd.dma_start(embag_in[:], row_sb[0:1, :])
        nc.gpsimd.collective_compute("AllGather", ALU.bypass,
                                     replica_groups=RG,
                                     ins=[embag_in[:].opt()],
                                     outs=[embag_out[:].opt()])
        embag2d = embag_out[:].rearrange("(c e) -> c e", e=E)

        # ================= conv1..3 =================
        pre_pad = sp.tile([6, 514], F32, tag="pre_pad")
        nc.vector.memset(pre_pad[:], 0.0)
        dma(pre_pad[0:4, 1:513], h_i.ap())
        nc.gpsimd.indirect_dma_start(
            pre_pad[4:6, 1:513], None, embag2d,
            bass.IndirectOffsetOnAxis(ap=own_sb[0:2, 0:1], axis=0))

        ps1 = ppA.tile([32, 512], F32, tag="ppA")
        conv3tap(ps1, pre_pad[0:5, :], cw1, 32)
        c1pad = sp.tile([32, 514], F32, tag="c1pad")
        nc.vector.memset(c1pad[:], 0.0)
        bn_relu(ps1[:], 32, bng["g1"][:], bng["b1"][:], c1pad[:, 1:513], "1")

        ps2 = ppA.tile([5, 512], F32, tag="ppA")
        conv3tap(ps2, c1pad[:], cw2, 5)
        res2 = sp.tile([5, 512], F32, tag="res2")
        nc.vector.tensor_add(res2[:], ps2[0:5, :], pre_pad[0:5, 1:513])
        pre2pad = sp.tile([5, 514], F32, tag="pre2pad")
        nc.vector.memset(pre2pad[:], 0.0)
        bn_relu(res2[:], 5, bng["g2"][:], bng["b2"][:], pre2pad[:, 1:513], "2")

        ps3 = ppA.tile([128, 4], F32, tag="ppA")
        convT(ps3, pre2pad[:], cw3)
        preT = sp.tile([128, 4], F32, tag="preT")
        bn_relu_T(ps3, bng["g3"][:], bng["b3"][:], preT, "3")

        # ================= attention =================
        ps_lg = ppA.tile([1, LS], F32, tag="ppA")
        for kc in range(4):
            nc.tensor.matmul(ps_lg[:], preT[:, kc:kc + 1],
                             attw[:, kc, :], start=(kc == 0), stop=(kc == 3))
        lg = sp.tile([1, LS], F32, tag="lg")
        nc.vector.tensor_add(lg[:], ps_lg[:], attb[:])
        nm_att = sp.tile([1, 1], F32, tag="nm_att")
        nc.vector.tensor_reduce(nm_att[:], lg[:], axis=AX.X, op=ALU.max,
                                negate=True)
        e_sb = sp.tile([1, LS], F32, tag="e_sb")
        s_att = sp.tile([1, 1], F32, tag="s_att")
        nc.scalar.activation(e_sb[:], lg[:], ACTF.Exp, bias=nm_att[:],
                             accum_out=s_att[:])
        ps_eT = ppA.tile([128, 2], F32, tag="ppA")
        for j in range(2):
            nc.tensor.transpose(ps_eT[:, j:j + 1],
                                e_sb[0:1, j * 128:(j + 1) * 128],
                                idt[0:1, 0:1])
        eT = sp.tile([128, 2], F32, tag="eT")
        nc.vector.tensor_copy(eT[:], ps_eT[:])
        ps_papp = ppB.tile([1, 1025], F32, tag="ppB")
        for nh in range(2):
            for j in range(2):
                nc.tensor.matmul(ps_papp[0:1, nh * 512:(nh + 1) * 512],
                                 eT[:, j:j + 1],
                                 [enc0, enc1][j][:, nh * 512:(nh + 1) * 512],
                                 start=(j == 0), stop=(j == 1))
        pkg = sp.tile([1, 1032], F32, tag="pkg")
        nc.vector.memset(pkg[:], 0.0)
        nc.scalar.copy(pkg[0:1, 0:1024], ps_papp[0:1, 0:1024])
        nc.vector.tensor_copy(pkg[0:1, 1024:1025], nm_att[:])
        nc.vector.tensor_copy(pkg[0:1, 1025:1026], s_att[:])
        attag_in = dp.tile([1032], F32, tag="attag_in")
        attag_out = dp.tile([NCORE * 1032], F32, tag="attag_out")
        nc.gpsimd.dma_start(attag_in[:], pkg[:])
        nc.gpsimd.collective_compute("AllGather", ALU.bypass,
                                     replica_groups=RG,
                                     ins=[attag_in[:].opt()],
                                     outs=[attag_out[:].opt()])
        pkg8 = sp.tile([NCORE, 1032], F32, tag="pkg8")
        dma(pkg8[:], attag_out[:].rearrange("(c e) -> c e", e=1032))
        # global max over cores: m = -min(nm); scale_o = exp(-nm_o - M)
        negM = part_reduce(pkg8[:, 1024:1025], ALU.min, "att_negM",
                           npart=NCORE)
        negM_bc = bc128(negM[:], "att_negM_bc", npart=NCORE)
        scale8 = sp.tile([NCORE, 1], F32, tag="scale8")
        nc.scalar.activation(scale8[:], pkg8[:, 1024:1025], ACTF.Exp,
                             bias=negM_bc[:], scale=-1.0)
        ps_app = ppB.tile([1, 1025], F32, tag="ppB")
        for nh in range(2):
            nc.tensor.matmul(ps_app[0:1, nh * 512:(nh + 1) * 512],
                             scale8[:], pkg8[:, nh * 512:(nh + 1) * 512],
                             start=True, stop=True)
        nc.tensor.matmul(ps_app[0:1, 1024:1025], scale8[:],
                         pkg8[:, 1025:1026], start=True, stop=True)
        stot = sp.tile([1, 1], F32, tag="stot")
        nc.vector.tensor_copy(stot[:], ps_app[0:1, 1024:1025])
        rcp = sp.tile([1, 1], F32, tag="rcp")
        nc.vector.reciprocal(rcp[:], stot[:])
        app = sp.tile([1, 1024], F32, tag="app")
        nc.vector.tensor_scalar(app[:], ps_app[0:1, 0:1024], rcp[:], None,
                                ALU.mult)

        # ================= conv4..6 =================
        com_pad = sp.tile([4, 514], F32, tag="com_pad")
        nc.vector.memset(com_pad[:], 0.0)
        dma(com_pad[0:2, 1:513], app[:])
        nc.gpsimd.indirect_dma_start(
            com_pad[2:4, 1:513], None, embag2d,
            bass.IndirectOffsetOnAxis(ap=own_sb[0:2, 0:1], axis=0))

        ps4 = ppA.tile([32, 512], F32, tag="ppA")
        conv3tap(ps4, com_pad[0:3, :], cw4, 32)
        c4pad = sp.tile([32, 514], F32, tag="c4pad")
        nc.vector.memset(c4pad[:], 0.0)
        bn_relu(ps4[:], 32, bng["g4"][:], bng["b4"][:], c4pad[:, 1:513], "4")

        ps5 = ppA.tile([3, 512], F32, tag="ppA")
        conv3tap(ps5, c4pad[:], cw5, 3)
        res5 = sp.tile([3, 512], F32, tag="res5")
        nc.vector.tensor_add(res5[:], ps5[0:3, :], com_pad[0:3, 1:513])
        com2pad = sp.tile([3, 514], F32, tag="com2pad")
        nc.vector.memset(com2pad[:], 0.0)
        bn_relu(res5[:], 3, bng["g5"][:], bng["b5"][:], com2pad[:, 1:513], "5")

        ps6 = ppA.tile([128, 4], F32, tag="ppA")
        convT(ps6, com2pad[:], cw6)
        xtT = sp.tile([128, 4], F32, tag="xtT0")
        bn_relu_T(ps6, bng["g6"][:], bng["b6"][:], xtT, "6")

        # ================= GRU (4 layers, output-sharded) =================
        xt_ag_outs = []
        for l in range(4):
            ps_gi = ppA.tile([1, 192], F32, tag="ppA")
            ps_gh = ppA.tile([1, 192], F32, tag="ppA")
            for kc in range(4):
                nc.tensor.matmul(ps_gi[:], xtT[:, kc:kc + 1],
                                 wih[:, l, kc, :],
                                 start=(kc == 0), stop=(kc == 3))
                nc.tensor.matmul(ps_gh[:], hT[:, kc * 4 + l:kc * 4 + l + 1],
                                 whh[:, l, kc, :],
                                 start=(kc == 0), stop=(kc == 3))
            gi = sp.tile([1, 192], F32, tag="gi")
            gh = sp.tile([1, 192], F32, tag="gh")
            nc.vector.tensor_add(gi[:], ps_gi[:], bih[0:1, l, :])
            nc.vector.tensor_add(gh[:], ps_gh[:], bhh[0:1, l, :])
            rz = sp.tile([1, 128], F32, tag="rz")
            nc.vector.tensor_add(rz[:], gi[0:1, 0:128], gh[0:1, 0:128])
            sig = sp.tile([1, 128], F32, tag="sig")
            nc.scalar.activation(sig[:], rz[:], ACTF.Sigmoid)
            nt = sp.tile([1, G], F32, tag="nt")
            nc.vector.tensor_mul(nt[:], sig[0:1, 0:G], gh[0:1, 128:192])
            nc.vector.tensor_add(nt[:], nt[:], gi[0:1, 128:192])
            n2 = sp.tile([1, G], F32, tag="n2")
            nc.scalar.activation(n2[:], nt[:], ACTF.Tanh)
            d_ = sp.tile([1, G], F32, tag="d_")
            nc.vector.tensor_sub(d_[:], hprev[0:1, l * G:(l + 1) * G], n2[:])
            nc.vector.tensor_mul(d_[:], sig[0:1, G:128], d_[:])
            hnew = sp.tile([1, G], F32, tag="hnew")
            nc.vector.tensor_add(hnew[:], n2[:], d_[:])
            ag_in = dp.tile([G], F32, tag=f"xtag_in{l}")
            ag_out = dp.tile([H], F32, tag=f"xtag_out{l}")
            nc.gpsimd.dma_start(ag_in[:], hnew[:])
            nc.gpsimd.collective_compute("AllGather", ALU.bypass,
                                         replica_groups=RG,
                                         ins=[ag_in[:].opt()],
                                         outs=[ag_out[:].opt()])
            xt_ag_outs.append(ag_out)
            xt_sb = sp.tile([1, H], F32, tag=f"xt_sb{l}")
            dma(xt_sb[:], ag_out[:].rearrange("(a e) -> a e", a=1))
            ps_xtT = ppA.tile([128, 4], F32, tag="ppA")
            for j in range(4):
                nc.tensor.transpose(ps_xtT[:, j:j + 1],
                                    xt_sb[0:1, j * 128:(j + 1) * 128],
                                    idt[0:1, 0:1])
            xtT = sp.tile([128, 4], F32, tag=f"xtT{l + 1}")
            nc.vector.tensor_copy(xtT[:], ps_xtT[:])

        for l in range(4):
            dma(out_h.ap()[l:l + 1, :],
                xt_ag_outs[l][:].rearrange("(a e) -> a e", a=1))

        # ================= output projection + log_softmax =================
        ps_lo = ppLO.tile([128, VT], F32, tag="ppLO")
        for vt in range(VT):
            for kc in range(4):
                nc.tensor.matmul(ps_lo[:, vt:vt + 1],
                                 low[:, (vt * 4 + kc) * 128:
                                     (vt * 4 + kc + 1) * 128],
                                 xtT[:, kc:kc + 1],
                                 start=(kc == 0), stop=(kc == 3))
        logits = sp.tile([128, VT], F32, tag="logits")
        nc.vector.tensor_add(logits[:], ps_lo[:], lob[:])
        maxc = sp.tile([128, 1], F32, tag="maxc")
        nc.vector.tensor_reduce(maxc[:], logits[:], axis=AX.X, op=ALU.max)
        negm = part_reduce(maxc[:], ALU.max, "lo_negm", negate=True)
        negm_bc = bc128(negm[:], "lo_negm_bc")
        ebuf = sp.tile([128, VT], F32, tag="ebuf")
        sumc = sp.tile([128, 1], F32, tag="sumc")
        nc.scalar.activation(ebuf[:], logits[:], ACTF.Exp, bias=negm_bc[:],
                             accum_out=sumc[:])
        s_loc = part_reduce(sumc[:], ALU.add, "lo_sloc")
        stat = sp.tile([1, 8], F32, tag="stat")
        nc.vector.memset(stat[:], 0.0)
        nc.vector.tensor_copy(stat[0:1, 0:1], negm[:])
        nc.vector.tensor_copy(stat[0:1, 1:2], s_loc[:])
        stag_in = dp.tile([8], F32, tag="stag_in")
        stag_out = dp.tile([NCORE * 8], F32, tag="stag_out")
        nc.gpsimd.dma_start(stag_in[:], stat[:])
        nc.gpsimd.collective_compute("AllGather", ALU.bypass,
                                     replica_groups=RG,
                                     ins=[stag_in[:].opt()],
                                     outs=[stag_out[:].opt()])
        st8 = sp.tile([NCORE, 8], F32, tag="st8")
        dma(st8[:], stag_out[:].rearrange("(c e) -> c e", e=8))
        negM2 = part_reduce(st8[:, 0:1], ALU.min, "lo_negM2", npart=NCORE)
        negM2_bc = bc128(negM2[:], "lo_negM2_bc", npart=NCORE)
        scl8 = sp.tile([NCORE, 1], F32, tag="scl8")
        nc.scalar.activation(scl8[:], st8[:, 0:1], ACTF.Exp,
                             bias=negM2_bc[:], scale=-1.0)
        ps_stot = ppT.tile([1, 1], F32, tag="ppT")
        nc.tensor.matmul(ps_stot[:], scl8[:], st8[:, 1:2],
                         start=True, stop=True)
        lnz = sp.tile([1, 1], F32, tag="lnz")
        nc.scalar.activation(lnz[:], ps_stot[:], ACTF.Ln)
        logz = sp.tile([1, 1], F32, tag="logz")
        nc.vector.tensor_sub(logz[:], lnz[:], negM2[:])
        logz_bc = bc128(logz[:], "logz_bc")
        outsb = sp.tile([128, VT], F32, tag="outsb")
        nc.vector.tensor_scalar(outsb[:], logits[:], logz_bc[:], None,
                                ALU.subtract)
        dma(out_lp.ap(), outsb[:])

    return nc


# ======================= host-side prep =======================

def prep_in_maps(inp):
    np32 = lambda a: np.ascontiguousarray(np.asarray(a), dtype=np.float32)
    emb = np32(inp["emb"])
    lo_w = np32(inp["lo_w"]); lo_b = np32(inp["lo_b"])
    att_w = np32(inp["att_w"]); att_b = np32(inp["att_b"])
    enc = np32(inp["encoder_outs"])
    h = np32(inp["h_state"]).reshape(4, H)
    x = int(np.asarray(inp["x"]).reshape(-1)[0])

    emb_pad = np.zeros((NCORE * V8, E), np.float32)
    emb_pad[:V] = emb
    lo_w_pad = np.zeros((NCORE * V8, H), np.float32)
    lo_w_pad[:V] = lo_w
    lo_b_pad = np.full((NCORE * V8,), NEGB, np.float32)
    lo_b_pad[:V] = lo_b

    # conv weights: lhsT layout [I, k, O]
    def cws(w):
        return np.ascontiguousarray(np32(w).transpose(1, 2, 0)).reshape(
            w.shape[1], -1)
    cw1 = cws(inp["conv1_w"]); cw2 = cws(inp["conv2_w"])
    cw4 = cws(inp["conv4_w"]); cw5 = cws(inp["conv5_w"])
    cw3 = np32(inp["conv3_w"])[0]          # (5,3)
    cw6 = np32(inp["conv6_w"])[0]          # (3,3)

    hT2 = np.ascontiguousarray(
        h.T.reshape(4, 128, 4).transpose(1, 0, 2)).reshape(128, 16)

    gwih = np32(inp["gru_wih"]); gwhh = np32(inp["gru_whh"])
    gbih = np32(inp["gru_bih"]); gbhh = np32(inp["gru_bhh"])

    common = dict(
        cw1_i=cw1, cw2_i=cw2, cw3_i=cw3, cw4_i=cw4, cw5_i=cw5, cw6_i=cw6,
        h_i=h, hT_i=hT2,
        idt_i=np.eye(128, dtype=np.float32),
    )
    for nm, key, c in [("1", "bn1", 32), ("2", "bn2", 5), ("3", "bn3", 1),
                       ("4", "bn4", 32), ("5", "bn5", 3), ("6", "bn6", 1)]:
        common[f"bng{nm}_i"] = np32(inp[key + "_g"]).reshape(c, 1)
        common[f"bnb{nm}_i"] = np32(inp[key + "_b"]).reshape(c, 1)

    maps = []
    for c in range(NCORE):
        m = dict(common)
        loc = min(max(x - c * V8, 0), V8 - 1)
        m["idx_i"] = np.array([[loc], [loc]], np.int32)
        m["own_i"] = np.array([[x // V8], [x // V8]], np.int32)
        m["emb_i"] = np.ascontiguousarray(emb_pad[c * V8:(c + 1) * V8])
        rows = np.r_[c * G:(c + 1) * G, 512 + c * G:512 + (c + 1) * G,
                     1024 + c * G:1024 + (c + 1) * G]
        m["hprev_i"] = np.ascontiguousarray(
            h[:, c * G:(c + 1) * G]).reshape(1, 4 * G)
        wih_l = np.stack([
            np.ascontiguousarray(gwih[l][rows].T)      # (512,192)
            .reshape(4, 128, 192) for l in range(4)])  # (4,4,128,192)
        m["wih_i"] = np.ascontiguousarray(
            wih_l.transpose(2, 0, 1, 3)).reshape(128, 16 * 192)
        whh_l = np.stack([
            np.ascontiguousarray(gwhh[l][rows].T).reshape(4, 128, 192)
            for l in range(4)])
        m["whh_i"] = np.ascontiguousarray(
            whh_l.transpose(2, 0, 1, 3)).reshape(128, 16 * 192)
        m["bih_i"] = np.ascontiguousarray(gbih[:, rows]).reshape(1, 768)
        m["bhh_i"] = np.ascontiguousarray(gbhh[:, rows]).reshape(1, 768)
        aw = att_w[c * LS:(c + 1) * LS]                # (256,512)
        m["attw_i"] = np.ascontiguousarray(
            aw.T.reshape(4, 128, LS).transpose(1, 0, 2)).reshape(128, 4 * LS)
        m["attb_i"] = np.ascontiguousarray(
            att_b[c * LS:(c + 1) * LS]).reshape(1, LS)
        m["enc_i"] = np.ascontiguousarray(enc[c * LS:(c + 1) * LS])
        wsh = lo_w_pad[c * V8:(c + 1) * V8]            # (6656,512)
        m["low_i"] = np.ascontiguousarray(
            wsh.reshape(VT, 128, 4, 128).transpose(3, 0, 2, 1)).reshape(
                128, VT * 512)
        m["lob_i"] = np.ascontiguousarray(
            lo_b_pad[c * V8:(c + 1) * V8].reshape(VT, 128).T)
        maps.append(m)
    return maps


_CACHE = {}


def get_compiled():
    if "nc" not in _CACHE:
        nc = build_nc()
        nc.compile()
        _CACHE["nc"] = nc
    return _CACHE["nc"]


def _install_ntff_hook():
    """Provide antenv.axon_hooks (absent in this image) so trace=True works."""
    import types

    if "antenv.axon_hooks" in sys.modules:
        return
    mod = types.ModuleType("antenv.axon_hooks")
    state = {}
    mod.set_axon_ntff_profile_hook = lambda h: state.__setitem__("h", h)
    mod.get_axon_ntff_profile_hook = lambda: state.get("h")
    sys.modules["antenv.axon_hooks"] = mod
    try:
        if "/root/.axon_site" not in sys.path:
            sys.path.insert(0, "/root/.axon_site")
        from trn_agent_boot.trn_boot import _ntff_profile_via_ctypes
        state["h"] = _ntff_profile_via_ctypes("/opt/axon/libaxon_pjrt.so")
    except Exception as e:  # degrade to no-trace
        print("ntff hook install failed:", e)
    import concourse.bass_utils as bu
    bu.upload_artifacts = lambda d: d


def run(inputs, trace=False):
    if trace:
        _install_ntff_hook()
    nc = get_compiled()
    in_maps = prep_in_maps(inputs)
    res = run_bass_kernel_spmd(nc, in_maps, core_ids=list(range(NCORE)),
                               trace=trace)
    outs = res.results
    lp = np.concatenate(
        [outs[c]["out_lp"].T.reshape(-1) for c in range(NCORE)])[:V]
    h_new = outs[0]["out_h"].reshape(4, 1, H).astype(np.float32)
    return (lp.reshape(1, V).astype(np.float32), h_new), res


def kernel(**inputs):
    out, _ = run(inputs)
    return out


# revision 18
# speedup vs baseline: 1.4379x; 1.4379x over previous
"""Trainium2 Bass kernel for nn_Attention_Decoder (8-core tensor-parallel).

Sharding:
  - emb / lo_w / lo_b: vocab-sharded (V8=6656 rows/core, padded).
  - att_w / att_b / encoder_outs: L-sharded (256 rows/core), softmax via
    log-sum-exp package AllGather.
  - GRU: output-sharded (64 h-elems/core), per-layer AllGather of h chunk.
  - conv/BN stack: replicated on every core (tiny).
Collectives (all 8-core AllGather): emb row, attention package, 4x GRU xt,
logit stats.
"""
import sys
import numpy as np
import ml_dtypes

sys.path.insert(0, "/opt/trn_rl_repo")

import concourse.bass as bass
import concourse.bacc as bacc
import concourse.tile as tile
from concourse import mybir
from concourse.bass_utils import run_bass_kernel_spmd

F32 = mybir.dt.float32
I32 = mybir.dt.int32
AX = mybir.AxisListType
ALU = mybir.AluOpType
ACTF = mybir.ActivationFunctionType

NCORE = 8
V = 50257
E = H = 512
LATT = 2048
V8 = 6656            # padded vocab rows per core
VT = V8 // 128       # 52
LS = LATT // NCORE   # 256
G = H // NCORE       # 64
EPS = 1e-5
NEGB = -1e30
RG = [[0, 1, 2, 3, 4, 5, 6, 7]]


def build_nc():
    nc = bacc.Bacc("TRN2", target_bir_lowering=False, debug=False,
                   num_devices=NCORE)

    def din(name, shape, dtype=F32):
        return nc.dram_tensor(name, list(shape), dtype, kind="ExternalInput")

    # ---- inputs (per-core values supplied in in_maps) ----
    idx_i = din("idx_i", (2, 1), I32)        # clamp(x - c*V8, 0, V8-1), dup
    own_i = din("own_i", (2, 1), I32)        # owner core = x // V8, dup
    emb_i = din("emb_i", (V8, E))            # emb vocab shard
    h_i = din("h_i", (4, H))                 # h_state rows
    hT_i = din("hT_i", (128, 16))            # [p, kc*4+l] = h[l, kc*128+p]
    hprev_i = din("hprev_i", (1, 4 * G))     # [l*G+j] = h[l, c*G+j]
    cw1_i = din("cw1_i", (5, 3 * 32))
    cw2_i = din("cw2_i", (32, 3 * 5))
    cw3_i = din("cw3_i", (5, 3))
    cw4_i = din("cw4_i", (3, 3 * 32))
    cw5_i = din("cw5_i", (32, 3 * 3))
    cw6_i = din("cw6_i", (3, 3))
    bng1_i = din("bng1_i", (32, 1)); bnb1_i = din("bnb1_i", (32, 1))
    bng2_i = din("bng2_i", (5, 1)); bnb2_i = din("bnb2_i", (5, 1))
    bng3_i = din("bng3_i", (1, 1)); bnb3_i = din("bnb3_i", (1, 1))
    bng4_i = din("bng4_i", (32, 1)); bnb4_i = din("bnb4_i", (32, 1))
    bng5_i = din("bng5_i", (3, 1)); bnb5_i = din("bnb5_i", (3, 1))
    bng6_i = din("bng6_i", (1, 1)); bnb6_i = din("bnb6_i", (1, 1))
    attw_i = din("attw_i", (128, 4 * LS))    # [p, kc*LS+j] = att_w[c*LS+j, kc*128+p]
    attb_i = din("attb_i", (1, LS))
    enc_i = din("enc_i", (LS, 2 * H))        # encoder_outs shard rows
    wih_i = din("wih_i", (128, 16 * 192))    # [p,(l*4+kc)*192+r]
    whh_i = din("whh_i", (128, 16 * 192))
    bih_i = din("bih_i", (1, 4 * 192))
    bhh_i = din("bhh_i", (1, 4 * 192))
    BF16 = mybir.dt.bfloat16
    low_i = din("low_i", (128, VT * 512), BF16)  # [k,(vt*4+kc)*128+m] = W[vt*128+m, kc*128+k]
    lob_i = din("lob_i", (128, VT))          # [p, vt] = b[vt*128+p]
    idt_i = din("idt_i", (128, 128))

    out_lp = nc.dram_tensor("out_lp", [128, VT], F32, kind="ExternalOutput")
    out_h = nc.dram_tensor("out_h", [4, H], F32, kind="ExternalOutput")

    with tile.TileContext(nc) as tc, \
         tc.tile_pool(name="w", bufs=1) as wp, \
         tc.tile_pool(name="s", bufs=1) as sp, \
         tc.tile_pool(name="ppA", bufs=3, space="PSUM") as ppA, \
         tc.tile_pool(name="ppT", bufs=1, space="PSUM") as ppT, \
         tc.tile_pool(name="ppB", bufs=1, space="PSUM") as ppB, \
         tc.tile_pool(name="ppLO", bufs=1, space="PSUM") as ppLO, \
         tc.tile_pool(name="dram", bufs=1, space="DRAM") as dp:

        dma = nc.sync.dma_start

        def wtile(src, shape, tag, dt=F32):
            t = wp.tile(list(shape), dt, tag=tag)
            dma(t[:], src.ap())
            return t

        # ---- load small weights (critical-path inputs first) ----
        idx_sb = sp.tile([2, 1], I32, tag="idx_sb")
        own_sb = sp.tile([2, 1], I32, tag="own_sb")
        dma(idx_sb[:], idx_i.ap())
        dma(own_sb[:], own_i.ap())
        idt = wtile(idt_i, (128, 128), "idt")
        h_sb = wtile(h_i, (4, H), "h_sb")
        hT = wtile(hT_i, (128, 16), "hT")
        hprev = wtile(hprev_i, (1, 4 * G), "hprev")
        cw1 = wtile(cw1_i, (5, 3, 32), "cw1")
        cw2 = wtile(cw2_i, (32, 3, 5), "cw2")
        cw3 = wtile(cw3_i, (5, 3), "cw3")
        cw4 = wtile(cw4_i, (3, 3, 32), "cw4")
        cw5 = wtile(cw5_i, (32, 3, 3), "cw5")
        cw6 = wtile(cw6_i, (3, 3), "cw6")
        bng = {}
        for nm, hd, c in [("g1", bng1_i, 32), ("b1", bnb1_i, 32),
                          ("g2", bng2_i, 5), ("b2", bnb2_i, 5),
                          ("g3", bng3_i, 1), ("b3", bnb3_i, 1),
                          ("g4", bng4_i, 32), ("b4", bnb4_i, 32),
                          ("g5", bng5_i, 3), ("b5", bnb5_i, 3),
                          ("g6", bng6_i, 1), ("b6", bnb6_i, 1)]:
            bng[nm] = wtile(hd, (c, 1), "bn" + nm)
        attw = wtile(attw_i, (128, 4, LS), "attw")
        attb = wtile(attb_i, (1, LS), "attb")
        enc0 = wp.tile([128, 2 * H], F32, tag="enc0")
        enc1 = wp.tile([128, 2 * H], F32, tag="enc1")
        dma(enc0[:], enc_i.ap()[0:128, :])
        dma(enc1[:], enc_i.ap()[128:256, :])
        wih = wtile(wih_i, (128, 4, 4, 192), "wih")
        whh = wtile(whh_i, (128, 4, 4, 192), "whh")
        bih = wtile(bih_i, (1, 4, 192), "bih")
        bhh = wtile(bhh_i, (1, 4, 192), "bhh")
        lob = wtile(lob_i, (128, VT), "lob")
        low = wp.tile([128, VT * 512], BF16, tag="low")
        for g in range(13):
            dma(low[:, g * 2048:(g + 1) * 2048],
                low_i.ap()[:, g * 2048:(g + 1) * 2048])

        ones_row = wp.tile([1, 128], F32, tag="ones_row")
        nc.vector.memset(ones_row[:], 1.0)

        # ---- helpers ----
        def bc128(src11, tag, npart=128):
            """materialize [npart,1] broadcast of a [1,1] sbuf value via PE"""
            ps = ppT.tile([npart, 1], F32, tag="ppT")
            nc.tensor.matmul(ps[:], ones_row[0:1, 0:npart], src11,
                             start=True, stop=True)
            sb = sp.tile([npart, 1], F32, tag=tag)
            nc.vector.tensor_copy(sb[:], ps[:])
            return sb

        def part_reduce(col_sb, op, tag, npart=128, negate=False):
            """reduce a [npart,1] sbuf column across partitions -> [1,1] sbuf"""
            ps = ppT.tile([1, npart], F32, tag="ppT")
            nc.tensor.transpose(ps[:], col_sb, idt[0:npart, 0:npart])
            out = sp.tile([1, 1], F32, tag=tag)
            nc.vector.tensor_reduce(out[:], ps[:], axis=AX.X, op=op,
                                    negate=negate)
            return out

        def conv3tap(psum, xpad, w_sb, O):
            for k in range(3):
                nc.tensor.matmul(psum[0:O, :], w_sb[:, k, :],
                                 xpad[:, k:k + 512],
                                 start=(k == 0), stop=(k == 2))

        def convT(psum128x4, xpad, w_sb):
            for c4 in range(4):
                for k in range(3):
                    nc.tensor.matmul(psum128x4[:, c4:c4 + 1],
                                     xpad[:, c4 * 128 + k:c4 * 128 + k + 128],
                                     w_sb[:, k:k + 1],
                                     start=(k == 0), stop=(k == 2))

        def bn_relu(src, C, g_sb, b_sb, out_ap, uid):
            """src [C,512] (psum) -> out_ap [C,512] (sbuf), training-mode BN."""
            sq = sp.tile([32, 512], F32, tag="bn_sq")
            sumsq = sp.tile([32, 1], F32, tag="bn_ss")
            sx = sp.tile([32, 1], F32, tag="bn_sx")
            nc.scalar.activation(sq[0:C, :], src, ACTF.Square,
                                 accum_out=sumsq[0:C, :])
            nc.vector.tensor_reduce(sx[0:C, :], src, axis=AX.X, op=ALU.add)
            mean = sp.tile([32, 1], F32, tag="bn_m")
            ex2 = sp.tile([32, 1], F32, tag="bn_e")
            nc.vector.tensor_scalar(mean[0:C, :], sx[0:C, :], 1.0 / 512, None,
                                    ALU.mult)
            nc.vector.tensor_scalar(ex2[0:C, :], sumsq[0:C, :], 1.0 / 512,
                                    None, ALU.mult)
            var = sp.tile([32, 1], F32, tag="bn_v")
            m2 = sp.tile([32, 1], F32, tag="bn_m2")
            nc.vector.tensor_mul(m2[0:C, :], mean[0:C, :], mean[0:C, :])
            nc.vector.tensor_sub(var[0:C, :], ex2[0:C, :], m2[0:C, :])
            nc.vector.tensor_scalar(var[0:C, :], var[0:C, :], EPS, None,
                                    ALU.add)
            std = sp.tile([32, 1], F32, tag="bn_st")
            nc.scalar.activation(std[0:C, :], var[0:C, :], ACTF.Sqrt)
            rstd = sp.tile([32, 1], F32, tag="bn_rs")
            nc.vector.reciprocal(rstd[0:C, :], std[0:C, :])
            s_ = sp.tile([32, 1], F32, tag="bn_s_")
            t_ = sp.tile([32, 1], F32, tag="bn_t_")
            nc.vector.tensor_mul(s_[0:C, :], rstd[0:C, :], g_sb)
            nc.vector.tensor_mul(t_[0:C, :], mean[0:C, :], s_[0:C, :])
            nc.vector.tensor_sub(t_[0:C, :], b_sb, t_[0:C, :])
            nc.scalar.activation(out_ap, src, ACTF.Relu,
                                 bias=t_[0:C, :], scale=s_[0:C, :])

        def bn_relu_T(psum, g11, b11, out_sb, uid):
            """src psum [128,4] (512 vals), BN over all, relu -> out_sb [128,4]"""
            sq = sp.tile([128, 4], F32, tag="bnT_sq")
            sumsq = sp.tile([128, 1], F32, tag="bnT_ss")
            sx = sp.tile([128, 1], F32, tag="bnT_sx")
            nc.scalar.activation(sq[:], psum[:], ACTF.Square,
                                 accum_out=sumsq[:])
            nc.vector.tensor_reduce(sx[:], psum[:], axis=AX.X, op=ALU.add)
            sxs = part_reduce(sx[:], ALU.add, "bnT_sxs_" + uid)
            sqs = part_reduce(sumsq[:], ALU.add, "bnT_sqs_" + uid)
            mean = sp.tile([1, 1], F32, tag="bnT_m")
            ex2 = sp.tile([1, 1], F32, tag="bnT_e")
            nc.vector.tensor_scalar(mean[:], sxs[:], 1.0 / 512, None, ALU.mult)
            nc.vector.tensor_scalar(ex2[:], sqs[:], 1.0 / 512, None, ALU.mult)
            m2 = sp.tile([1, 1], F32, tag="bnT_m2")
            var = sp.tile([1, 1], F32, tag="bnT_v")
            nc.vector.tensor_mul(m2[:], mean[:], mean[:])
            nc.vector.tensor_sub(var[:], ex2[:], m2[:])
            nc.vector.tensor_scalar(var[:], var[:], EPS, None, ALU.add)
            std = sp.tile([1, 1], F32, tag="bnT_st")
            nc.scalar.activation(std[:], var[:], ACTF.Sqrt)
            rstd = sp.tile([1, 1], F32, tag="bnT_rs")
            nc.vector.reciprocal(rstd[:], std[:])
            s11 = sp.tile([1, 1], F32, tag="bnT_s_")
            t11 = sp.tile([1, 1], F32, tag="bnT_t_")
            nc.vector.tensor_mul(s11[:], rstd[:], g11)
            nc.vector.tensor_mul(t11[:], mean[:], s11[:])
            nc.vector.tensor_sub(t11[:], b11, t11[:])
            s_bc = bc128(s11[:], "bnT_sbc_" + uid)
            t_bc = bc128(t11[:], "bnT_tbc_" + uid)
            nc.scalar.activation(out_sb[:], psum[:], ACTF.Relu,
                                 bias=t_bc[:], scale=s_bc[:])

        # ================= emb gather + AllGather =================
        row_sb = sp.tile([2, E], F32, tag="row_sb")
        nc.gpsimd.indirect_dma_start(
            row_sb[:], None, emb_i.ap(),
            bass.IndirectOffsetOnAxis(ap=idx_sb[0:2, 0:1], axis=0))
        embag_in = dp.tile([E], F32, tag="embag_in")
        embag_out = dp.tile([NCORE * E], F32, tag="embag_out")
        nc.gpsimd.dma_start(embag_in[:], row_sb[0:1, :])
        nc.gpsimd.collective_compute("AllGather", ALU.bypass,
                                     replica_groups=RG,
                                     ins=[embag_in[:].opt()],
                                     outs=[embag_out[:].opt()])
        embag2d = embag_out[:].rearrange("(c e) -> c e", e=E)

        # ================= conv1..3 =================
        pre_pad = sp.tile([6, 514], F32, tag="pre_pad")
        nc.vector.memset(pre_pad[:], 0.0)
        dma(pre_pad[0:4, 1:513], h_i.ap())
        nc.gpsimd.indirect_dma_start(
            pre_pad[4:6, 1:513], None, embag2d,
            bass.IndirectOffsetOnAxis(ap=own_sb[0:2, 0:1], axis=0))

        ps1 = ppA.tile([32, 512], F32, tag="ppA")
        conv3tap(ps1, pre_pad[0:5, :], cw1, 32)
        c1pad = sp.tile([32, 514], F32, tag="c1pad")
        nc.vector.memset(c1pad[:], 0.0)
        bn_relu(ps1[:], 32, bng["g1"][:], bng["b1"][:], c1pad[:, 1:513], "1")

        ps2 = ppA.tile([5, 512], F32, tag="ppA")
        conv3tap(ps2, c1pad[:], cw2, 5)
        res2 = sp.tile([5, 512], F32, tag="res2")
        nc.vector.tensor_add(res2[:], ps2[0:5, :], pre_pad[0:5, 1:513])
        pre2pad = sp.tile([5, 514], F32, tag="pre2pad")
        nc.vector.memset(pre2pad[:], 0.0)
        bn_relu(res2[:], 5, bng["g2"][:], bng["b2"][:], pre2pad[:, 1:513], "2")

        ps3 = ppA.tile([128, 4], F32, tag="ppA")
        convT(ps3, pre2pad[:], cw3)
        preT = sp.tile([128, 4], F32, tag="preT")
        bn_relu_T(ps3, bng["g3"][:], bng["b3"][:], preT, "3")

        # ================= attention =================
        ps_lg = ppA.tile([1, LS], F32, tag="ppA")
        for kc in range(4):
            nc.tensor.matmul(ps_lg[:], preT[:, kc:kc + 1],
                             attw[:, kc, :], start=(kc == 0), stop=(kc == 3))
        lg = sp.tile([1, LS], F32, tag="lg")
        nc.vector.tensor_add(lg[:], ps_lg[:], attb[:])
        nm_att = sp.tile([1, 1], F32, tag="nm_att")
        nc.vector.tensor_reduce(nm_att[:], lg[:], axis=AX.X, op=ALU.max,
                                negate=True)
        e_sb = sp.tile([1, LS], F32, tag="e_sb")
        s_att = sp.tile([1, 1], F32, tag="s_att")
        nc.scalar.activation(e_sb[:], lg[:], ACTF.Exp, bias=nm_att[:],
                             accum_out=s_att[:])
        ps_eT = ppA.tile([128, 2], F32, tag="ppA")
        for j in range(2):
            nc.tensor.transpose(ps_eT[:, j:j + 1],
                                e_sb[0:1, j * 128:(j + 1) * 128],
                                idt[0:1, 0:1])
        eT = sp.tile([128, 2], F32, tag="eT")
        nc.vector.tensor_copy(eT[:], ps_eT[:])
        ps_papp = ppB.tile([1, 1025], F32, tag="ppB")
        for nh in range(2):
            for j in range(2):
                nc.tensor.matmul(ps_papp[0:1, nh * 512:(nh + 1) * 512],
                                 eT[:, j:j + 1],
                                 [enc0, enc1][j][:, nh * 512:(nh + 1) * 512],
                                 start=(j == 0), stop=(j == 1))
        pkg = sp.tile([1, 1032], F32, tag="pkg")
        nc.vector.memset(pkg[:], 0.0)
        nc.scalar.copy(pkg[0:1, 0:1024], ps_papp[0:1, 0:1024])
        nc.vector.tensor_copy(pkg[0:1, 1024:1025], nm_att[:])
        nc.vector.tensor_copy(pkg[0:1, 1025:1026], s_att[:])
        attag_in = dp.tile([1032], F32, tag="attag_in")
        attag_out = dp.tile([NCORE * 1032], F32, tag="attag_out")
        nc.gpsimd.dma_start(attag_in[:], pkg[:])
        nc.gpsimd.collective_compute("AllGather", ALU.bypass,
                                     replica_groups=RG,
                                     ins=[attag_in[:].opt()],
                                     outs=[attag_out[:].opt()])
        pkg8 = sp.tile([NCORE, 1032], F32, tag="pkg8")
        dma(pkg8[:], attag_out[:].rearrange("(c e) -> c e", e=1032))
        # global max over cores: m = -min(nm); scale_o = exp(-nm_o - M)
        negM = part_reduce(pkg8[:, 1024:1025], ALU.min, "att_negM",
                           npart=NCORE)
        negM_bc = bc128(negM[:], "att_negM_bc", npart=NCORE)
        scale8 = sp.tile([NCORE, 1], F32, tag="scale8")
        nc.scalar.activation(scale8[:], pkg8[:, 1024:1025], ACTF.Exp,
                             bias=negM_bc[:], scale=-1.0)
        ps_app = ppB.tile([1, 1025], F32, tag="ppB")
        for nh in range(2):
            nc.tensor.matmul(ps_app[0:1, nh * 512:(nh + 1) * 512],
                             scale8[:], pkg8[:, nh * 512:(nh + 1) * 512],
                             start=True, stop=True)
        nc.tensor.matmul(ps_app[0:1, 1024:1025], scale8[:],
                         pkg8[:, 1025:1026], start=True, stop=True)
        stot = sp.tile([1, 1], F32, tag="stot")
        nc.vector.tensor_copy(stot[:], ps_app[0:1, 1024:1025])
        rcp = sp.tile([1, 1], F32, tag="rcp")
        nc.vector.reciprocal(rcp[:], stot[:])
        app = sp.tile([1, 1024], F32, tag="app")
        nc.vector.tensor_scalar(app[:], ps_app[0:1, 0:1024], rcp[:], None,
                                ALU.mult)

        # ================= conv4..6 =================
        com_pad = sp.tile([4, 514], F32, tag="com_pad")
        nc.vector.memset(com_pad[:], 0.0)
        dma(com_pad[0:2, 1:513], app[:])
        nc.gpsimd.indirect_dma_start(
            com_pad[2:4, 1:513], None, embag2d,
            bass.IndirectOffsetOnAxis(ap=own_sb[0:2, 0:1], axis=0))

        ps4 = ppA.tile([32, 512], F32, tag="ppA")
        conv3tap(ps4, com_pad[0:3, :], cw4, 32)
        c4pad = sp.tile([32, 514], F32, tag="c4pad")
        nc.vector.memset(c4pad[:], 0.0)
        bn_relu(ps4[:], 32, bng["g4"][:], bng["b4"][:], c4pad[:, 1:513], "4")

        ps5 = ppA.tile([3, 512], F32, tag="ppA")
        conv3tap(ps5, c4pad[:], cw5, 3)
        res5 = sp.tile([3, 512], F32, tag="res5")
        nc.vector.tensor_add(res5[:], ps5[0:3, :], com_pad[0:3, 1:513])
        com2pad = sp.tile([3, 514], F32, tag="com2pad")
        nc.vector.memset(com2pad[:], 0.0)
        bn_relu(res5[:], 3, bng["g5"][:], bng["b5"][:], com2pad[:, 1:513], "5")

        ps6 = ppA.tile([128, 4], F32, tag="ppA")
        convT(ps6, com2pad[:], cw6)
        xtT = sp.tile([128, 4], F32, tag="xtT0")
        bn_relu_T(ps6, bng["g6"][:], bng["b6"][:], xtT, "6")

        # ================= GRU (4 layers, output-sharded) =================
        xt_ag_outs = []
        for l in range(4):
            ps_gi = ppA.tile([1, 192], F32, tag="ppA")
            ps_gh = ppA.tile([1, 192], F32, tag="ppA")
            for kc in range(4):
                nc.tensor.matmul(ps_gi[:], xtT[:, kc:kc + 1],
                                 wih[:, l, kc, :],
                                 start=(kc == 0), stop=(kc == 3))
                nc.tensor.matmul(ps_gh[:], hT[:, kc * 4 + l:kc * 4 + l + 1],
                                 whh[:, l, kc, :],
                                 start=(kc == 0), stop=(kc == 3))
            gi = sp.tile([1, 192], F32, tag="gi")
            gh = sp.tile([1, 192], F32, tag="gh")
            nc.vector.tensor_add(gi[:], ps_gi[:], bih[0:1, l, :])
            nc.vector.tensor_add(gh[:], ps_gh[:], bhh[0:1, l, :])
            rz = sp.tile([1, 128], F32, tag="rz")
            nc.vector.tensor_add(rz[:], gi[0:1, 0:128], gh[0:1, 0:128])
            sig = sp.tile([1, 128], F32, tag="sig")
            nc.scalar.activation(sig[:], rz[:], ACTF.Sigmoid)
            nt = sp.tile([1, G], F32, tag="nt")
            nc.vector.tensor_mul(nt[:], sig[0:1, 0:G], gh[0:1, 128:192])
            nc.vector.tensor_add(nt[:], nt[:], gi[0:1, 128:192])
            n2 = sp.tile([1, G], F32, tag="n2")
            nc.scalar.activation(n2[:], nt[:], ACTF.Tanh)
            d_ = sp.tile([1, G], F32, tag="d_")
            nc.vector.tensor_sub(d_[:], hprev[0:1, l * G:(l + 1) * G], n2[:])
            nc.vector.tensor_mul(d_[:], sig[0:1, G:128], d_[:])
            hnew = sp.tile([1, G], F32, tag="hnew")
            nc.vector.tensor_add(hnew[:], n2[:], d_[:])
            ag_in = dp.tile([G], F32, tag=f"xtag_in{l}")
            ag_out = dp.tile([H], F32, tag=f"xtag_out{l}")
            nc.gpsimd.dma_start(ag_in[:], hnew[:])
            nc.gpsimd.collective_compute("AllGather", ALU.bypass,
                                         replica_groups=RG,
                                         ins=[ag_in[:].opt()],
                                         outs=[ag_out[:].opt()])
            xt_ag_outs.append(ag_out)
            xt4 = sp.tile([4, 128], F32, tag=f"xt4_{l}")
            dma(xt4[:], ag_out[:].rearrange("(a e) -> a e", a=4))
            ps_xtT = ppA.tile([128, 4], F32, tag="ppA")
            nc.tensor.transpose(ps_xtT[:], xt4[:], idt[0:4, 0:4])
            xtT = sp.tile([128, 4], BF16 if l == 3 else F32,
                          tag=f"xtT{l + 1}")
            nc.vector.tensor_copy(xtT[:], ps_xtT[:])

        for l in range(4):
            dma(out_h.ap()[l:l + 1, :],
                xt_ag_outs[l][:].rearrange("(a e) -> a e", a=1))

        # ================= output projection + log_softmax =================
        ps_lo = ppLO.tile([128, VT], F32, tag="ppLO")
        for vt in range(VT):
            for kc in range(4):
                nc.tensor.matmul(ps_lo[:, vt:vt + 1],
                                 low[:, (vt * 4 + kc) * 128:
                                     (vt * 4 + kc + 1) * 128],
                                 xtT[:, kc:kc + 1],
                                 start=(kc == 0), stop=(kc == 3))
        logits = sp.tile([128, VT], F32, tag="logits")
        nc.vector.tensor_add(logits[:], ps_lo[:], lob[:])
        maxc = sp.tile([128, 1], F32, tag="maxc")
        nc.vector.tensor_reduce(maxc[:], logits[:], axis=AX.X, op=ALU.max)
        negm = part_reduce(maxc[:], ALU.max, "lo_negm", negate=True)
        negm_bc = bc128(negm[:], "lo_negm_bc")
        ebuf = sp.tile([128, VT], F32, tag="ebuf")
        sumc = sp.tile([128, 1], F32, tag="sumc")
        nc.scalar.activation(ebuf[:], logits[:], ACTF.Exp, bias=negm_bc[:],
                             accum_out=sumc[:])
        s_loc = part_reduce(sumc[:], ALU.add, "lo_sloc")
        stat = sp.tile([1, 8], F32, tag="stat")
        nc.vector.memset(stat[:], 0.0)
        nc.vector.tensor_copy(stat[0:1, 0:1], negm[:])
        nc.vector.tensor_copy(stat[0:1, 1:2], s_loc[:])
        stag_in = dp.tile([8], F32, tag="stag_in")
        stag_out = dp.tile([NCORE * 8], F32, tag="stag_out")
        nc.gpsimd.dma_start(stag_in[:], stat[:])
        nc.gpsimd.collective_compute("AllGather", ALU.bypass,
                                     replica_groups=RG,
                                     ins=[stag_in[:].opt()],
                                     outs=[stag_out[:].opt()])
        st8 = sp.tile([NCORE, 8], F32, tag="st8")
        dma(st8[:], stag_out[:].rearrange("(c e) -> c e", e=8))
        negM2 = part_reduce(st8[:, 0:1], ALU.min, "lo_negM2", npart=NCORE)
        negM2_bc = bc128(negM2[:], "lo_negM2_bc", npart=NCORE)
        scl8 = sp.tile([NCORE, 1], F32, tag="scl8")
        nc.scalar.activation(scl8[:], st8[:, 0:1], ACTF.Exp,
                             bias=negM2_bc[:], scale=-1.0)
        ps_stot = ppT.tile([1, 1], F32, tag="ppT")
        nc.tensor.matmul(ps_stot[:], scl8[:], st8[:, 1:2],
                         start=True, stop=True)
        lnz = sp.tile([1, 1], F32, tag="lnz")
        nc.scalar.activation(lnz[:], ps_stot[:], ACTF.Ln)
        logz = sp.tile([1, 1], F32, tag="logz")
        nc.vector.tensor_sub(logz[:], lnz[:], negM2[:])
        logz_bc = bc128(logz[:], "logz_bc")
        outsb = sp.tile([128, VT], F32, tag="outsb")
        nc.vector.tensor_scalar(outsb[:], logits[:], logz_bc[:], None,
                                ALU.subtract)
        dma(out_lp.ap(), outsb[:])

    return nc


# ======================= host-side prep =======================

def prep_in_maps(inp):
    np32 = lambda a: np.ascontiguousarray(np.asarray(a), dtype=np.float32)
    emb = np32(inp["emb"])
    lo_w = np32(inp["lo_w"]); lo_b = np32(inp["lo_b"])
    att_w = np32(inp["att_w"]); att_b = np32(inp["att_b"])
    enc = np32(inp["encoder_outs"])
    h = np32(inp["h_state"]).reshape(4, H)
    x = int(np.asarray(inp["x"]).reshape(-1)[0])

    emb_pad = np.zeros((NCORE * V8, E), np.float32)
    emb_pad[:V] = emb
    lo_w_pad = np.zeros((NCORE * V8, H), np.float32)
    lo_w_pad[:V] = lo_w
    lo_b_pad = np.full((NCORE * V8,), NEGB, np.float32)
    lo_b_pad[:V] = lo_b

    # conv weights: lhsT layout [I, k, O]
    def cws(w):
        return np.ascontiguousarray(np32(w).transpose(1, 2, 0)).reshape(
            w.shape[1], -1)
    cw1 = cws(inp["conv1_w"]); cw2 = cws(inp["conv2_w"])
    cw4 = cws(inp["conv4_w"]); cw5 = cws(inp["conv5_w"])
    cw3 = np32(inp["conv3_w"])[0]          # (5,3)
    cw6 = np32(inp["conv6_w"])[0]          # (3,3)

    hT2 = np.ascontiguousarray(
        h.T.reshape(4, 128, 4).transpose(1, 0, 2)).reshape(128, 16)

    gwih = np32(inp["gru_wih"]); gwhh = np32(inp["gru_whh"])
    gbih = np32(inp["gru_bih"]); gbhh = np32(inp["gru_bhh"])

    common = dict(
        cw1_i=cw1, cw2_i=cw2, cw3_i=cw3, cw4_i=cw4, cw5_i=cw5, cw6_i=cw6,
        h_i=h, hT_i=hT2,
        idt_i=np.eye(128, dtype=np.float32),
    )
    for nm, key, c in [("1", "bn1", 32), ("2", "bn2", 5), ("3", "bn3", 1),
                       ("4", "bn4", 32), ("5", "bn5", 3), ("6", "bn6", 1)]:
        common[f"bng{nm}_i"] = np32(inp[key + "_g"]).reshape(c, 1)
        common[f"bnb{nm}_i"] = np32(inp[key + "_b"]).reshape(c, 1)

    maps = []
    for c in range(NCORE):
        m = dict(common)
        loc = min(max(x - c * V8, 0), V8 - 1)
        m["idx_i"] = np.array([[loc], [loc]], np.int32)
        m["own_i"] = np.array([[x // V8], [x // V8]], np.int32)
        m["emb_i"] = np.ascontiguousarray(emb_pad[c * V8:(c + 1) * V8])
        rows = np.r_[c * G:(c + 1) * G, 512 + c * G:512 + (c + 1) * G,
                     1024 + c * G:1024 + (c + 1) * G]
        m["hprev_i"] = np.ascontiguousarray(
            h[:, c * G:(c + 1) * G]).reshape(1, 4 * G)
        wih_l = np.stack([
            np.ascontiguousarray(gwih[l][rows].T)      # (512,192)
            .reshape(4, 128, 192) for l in range(4)])  # (4,4,128,192)
        m["wih_i"] = np.ascontiguousarray(
            wih_l.transpose(2, 0, 1, 3)).reshape(128, 16 * 192)
        whh_l = np.stack([
            np.ascontiguousarray(gwhh[l][rows].T).reshape(4, 128, 192)
            for l in range(4)])
        m["whh_i"] = np.ascontiguousarray(
            whh_l.transpose(2, 0, 1, 3)).reshape(128, 16 * 192)
        m["bih_i"] = np.ascontiguousarray(gbih[:, rows]).reshape(1, 768)
        m["bhh_i"] = np.ascontiguousarray(gbhh[:, rows]).reshape(1, 768)
        aw = att_w[c * LS:(c + 1) * LS]                # (256,512)
        m["attw_i"] = np.ascontiguousarray(
            aw.T.reshape(4, 128, LS).transpose(1, 0, 2)).reshape(128, 4 * LS)
        m["attb_i"] = np.ascontiguousarray(
            att_b[c * LS:(c + 1) * LS]).reshape(1, LS)
        m["enc_i"] = np.ascontiguousarray(enc[c * LS:(c + 1) * LS])
        wsh = lo_w_pad[c * V8:(c + 1) * V8]            # (6656,512)
        m["low_i"] = np.ascontiguousarray(
            wsh.reshape(VT, 128, 4, 128).transpose(3, 0, 2, 1)).reshape(
                128, VT * 512).astype(ml_dtypes.bfloat16)
        m["lob_i"] = np.ascontiguousarray(
            lo_b_pad[c * V8:(c + 1) * V8].reshape(VT, 128).T)
        maps.append(m)
    return maps


_CACHE = {}


def get_compiled():
    if "nc" not in _CACHE:
        nc = build_nc()
        nc.compile()
        _CACHE["nc"] = nc
    return _CACHE["nc"]


def _install_ntff_hook():
    """Provide antenv.axon_hooks (absent in this image) so trace=True works."""
    import types

    if "antenv.axon_hooks" in sys.modules:
        return
    mod = types.ModuleType("antenv.axon_hooks")
    state = {}
    mod.set_axon_ntff_profile_hook = lambda h: state.__setitem__("h", h)
    mod.get_axon_ntff_profile_hook = lambda: state.get("h")
    sys.modules["antenv.axon_hooks"] = mod
    try:
        if "/root/.axon_site" not in sys.path:
            sys.path.insert(0, "/root/.axon_site")
        from trn_agent_boot.trn_boot import _ntff_profile_via_ctypes
        state["h"] = _ntff_profile_via_ctypes("/opt/axon/libaxon_pjrt.so")
    except Exception as e:  # degrade to no-trace
        print("ntff hook install failed:", e)
    import concourse.bass_utils as bu
    bu.upload_artifacts = lambda d: d


def run(inputs, trace=False):
    if trace:
        _install_ntff_hook()
    nc = get_compiled()
    in_maps = prep_in_maps(inputs)
    res = run_bass_kernel_spmd(nc, in_maps, core_ids=list(range(NCORE)),
                               trace=trace)
    outs = res.results
    lp = np.concatenate(
        [outs[c]["out_lp"].T.reshape(-1) for c in range(NCORE)])[:V]
    h_new = outs[0]["out_h"].reshape(4, 1, H).astype(np.float32)
    return (lp.reshape(1, V).astype(np.float32), h_new), res


def kernel(**inputs):
    out, _ = run(inputs)
    return out
